# revision 1
# baseline (speedup 1.0000x reference)
"""Causal self-attention (B=2, S=2048, E=1024, H=16, DH=64) on 8 trn2 cores.

Sharding: core c -> (batch b = c//4, head-group g = c%4, heads 4g..4g+3).
Data parallel over batch, tensor parallel over heads, row-sharded Wo;
partial outputs summed on host.

Per-core device kernel (bf16 matmuls, fp32 accumulation):
  phase A: kqT = (x @ Wkq)^T via W-stationary matmuls on xT (+bias on DVE),
           v   =  x @ Wv  (natural layout, +ones column for row-sums)
  phase B: scores^T[sk,sq] = k q^T (2 heads row-packed in PE, K=64),
           P^T = exp(scores/8) (ACT only; causal-trimmed, triu-masked diag),
           AV: saT_aug = v_aug^T @ P^T (row 64 = softmax row-sums via the
           ones column); saT = (P^T V) * (1/rowsum) fused in the PSUM
           eviction (rowsum reciprocal broadcast via GPSIMD).
  phase C: out = saT^T @ Wo (row-shard), DMA to DRAM.
"""
import numpy as np
import ml_dtypes

import concourse.bass as bass
import concourse.bacc as bacc
import concourse.tile as tile
from concourse import mybir
from concourse.masks import make_upper_triangular

BF16 = mybir.dt.bfloat16
F32 = mybir.dt.float32
NP_BF16 = ml_dtypes.bfloat16

B, S, E, H, DH = 2, 2048, 1024, 16, 64
N_CORES = 8
HPC = 4          # heads per core
SCH = 4          # number of 512-wide sq chunks
SKT = 16         # number of 128-wide sk tiles
ET = 8           # number of 128-wide e tiles

Exp = mybir.ActivationFunctionType.Exp


def build_nc(reps=1):
    nc = bacc.Bacc(None, target_bir_lowering=False)

    xT = nc.dram_tensor("xT", [E, S], BF16, kind="ExternalInput")
    wkq = nc.dram_tensor("wkq", [E, 512], BF16, kind="ExternalInput")
    wv = nc.dram_tensor("wv", [E, 256], BF16, kind="ExternalInput")
    wo = nc.dram_tensor("wo", [256, E], BF16, kind="ExternalInput")
    bkq = nc.dram_tensor("bkq", [128, 4], F32, kind="ExternalInput")
    out = nc.dram_tensor("out", [S, E], F32, kind="ExternalOutput")

    with tile.TileContext(nc) as tc:
        import contextlib
        with contextlib.ExitStack() as ctx:
            const = ctx.enter_context(tc.tile_pool(name="const", bufs=1))
            wpool = ctx.enter_context(tc.tile_pool(name="wpool", bufs=1))
            xpool = ctx.enter_context(tc.tile_pool(name="xpool", bufs=1))
            kqpool = ctx.enter_context(tc.tile_pool(name="kqpool", bufs=1))
            vpool = ctx.enter_context(tc.tile_pool(name="vpool", bufs=1))
            sapool = ctx.enter_context(tc.tile_pool(name="sapool", bufs=1))
            pt_pool = ctx.enter_context(tc.tile_pool(name="pt", bufs=6))
            bc_pool = ctx.enter_context(tc.tile_pool(name="bc", bufs=4))
            small = ctx.enter_context(tc.tile_pool(name="small", bufs=8))
            ostage = ctx.enter_context(tc.tile_pool(name="ostage", bufs=6))

            bkq_sb = const.tile([128, 4], F32)
            nc.sync.dma_start(bkq_sb[:], bkq[:])
            triu2 = const.tile([128, 2, 128], BF16)
            make_upper_triangular(nc, triu2[:, 0, :], val=1.0, diag=True)
            make_upper_triangular(nc, triu2[:, 1, :], val=1.0, diag=True)

            # interleave weight/activation loads so the first kq matmuls can
            # start as soon as their e-tile arrives
            wkq_sb = wpool.tile([128, ET, 512], BF16)
            xT_sb = xpool.tile([128, ET, S], BF16)
            for e in range(ET):
                nc.sync.dma_start(wkq_sb[:, e, :], wkq[128 * e:128 * (e + 1), :])
                nc.sync.dma_start(xT_sb[:, e, :], xT[128 * e:128 * (e + 1), :])
            wv_sb = wpool.tile([128, ET, 256], BF16)
            nc.sync.dma_start(wv_sb[:], wv.rearrange("(n p) f -> p n f", p=128))
            wo_sb = wpool.tile([128, 2, E], BF16)
            nc.sync.dma_start(wo_sb[:], wo.rearrange("(n p) f -> p n f", p=128))

            kqT_sb = kqpool.tile([128, 4, S], BF16)      # blk: p0k,p0q,p1k,p1q
            vaug_sb = vpool.tile([128, SKT, HPC, 65], BF16)
            nc.vector.memset(vaug_sb[:, :, :, 64:65], 1.0)
            saT_sb = sapool.tile([128, 2, S], BF16)      # dim1: pair

            for _rep in range(reps):
              # All phases share one coexisting PSUM budget (8 banks:
              # kq 1 + v 1 + st 4 + av 2) and are emitted interleaved so the
              # scheduler can fill PE gaps in the ACT-bound attention loop
              # with projection matmuls.
              pools = {}
              def get_pool(name, bufs):
                  if name not in pools:
                      pools[name] = tc.alloc_tile_pool(name=name, bufs=bufs,
                                                       space="PSUM")
                  return pools[name]
              if True:
                get_pool("kq_ps", 3); get_pool("v_ps", 2)
                def kq_proj(blk):
                    for c in range(SCH):
                        ps = get_pool("kq_ps", 3).tile([128, 512], F32, tag="kqps")
                        for e in range(ET):
                            nc.tensor.matmul(
                                ps[:], wkq_sb[:, e, 128 * blk:128 * (blk + 1)],
                                xT_sb[:, e, 512 * c:512 * (c + 1)],
                                start=(e == 0), stop=(e == ET - 1))
                        nc.vector.tensor_scalar_add(
                            kqT_sb[:, blk, 512 * c:512 * (c + 1)], ps[:],
                            bkq_sb[:, blk:blk + 1])

                def v_proj(t0, t1):
                    for t in range(t0, t1):
                        ps = get_pool("v_ps", 2).tile([128, 256], F32)
                        for e in range(ET):
                            nc.tensor.matmul(
                                ps[:], xT_sb[:, e, 128 * t:128 * (t + 1)],
                                wv_sb[:, e, :],
                                start=(e == 0), stop=(e == ET - 1))
                        nc.vector.tensor_copy(
                            vaug_sb[:, t, :, 0:64],
                            ps[:].rearrange("p (h d) -> p h d", h=HPC))

                def attn_segment(c, p):
                    sq0 = 512 * c
                    kblk, qblk = 2 * p, 2 * p + 1
                    av = get_pool("av_ps", 2).tile([65, 2, 512], F32, tag="av")
                    nj = 4 * c + 4
                    for j in range(nj):
                        r = j - 4 * c
                        diag = r >= 0
                        off = 128 * r if diag else 0
                        w = 512 - off
                        st = get_pool("st_ps", 2).tile([128, 2, 512], F32, tag="st")
                        nc.tensor.matmul(
                            st[:, 0, 0:w],
                            kqT_sb[0:64, kblk, 128 * j:128 * (j + 1)],
                            kqT_sb[0:64, qblk, sq0 + off:sq0 + 512],
                            start=True, stop=True, tile_position=(0, 0))
                        nc.tensor.matmul(
                            st[:, 1, 0:w],
                            kqT_sb[64:128, kblk, 128 * j:128 * (j + 1)],
                            kqT_sb[64:128, qblk, sq0 + off:sq0 + 512],
                            start=True, stop=True, tile_position=(64, 0))
                        pt = pt_pool.tile([128, 2, 512], BF16, tag="pt")
                        nc.scalar.activation(pt[:, :, off:512],
                                             st[:, :, 0:w],
                                             Exp, scale=0.125)
                        if diag:
                            nc.vector.tensor_mul(
                                pt[:, :, off:off + 128],
                                pt[:, :, off:off + 128], triu2[:])
                        nc.tensor.matmul(av[:, 0, off:512],
                                         vaug_sb[:, j, 2 * p, :],
                                         pt[:, 0, off:512], start=(j == 0),
                                         stop=(j == nj - 1))
                        nc.tensor.matmul(av[:, 1, off:512],
                                         vaug_sb[:, j, 2 * p + 1, :],
                                         pt[:, 1, off:512], start=(j == 0),
                                         stop=(j == nj - 1))
                    # normalization + fused eviction for (p, c)
                    rs = small.tile([1, 2, 512], F32, tag="rs")
                    nc.vector.tensor_copy(rs[0:1, :, :], av[64:65, :, :])
                    rc = small.tile([1, 2, 512], F32, tag="rc")
                    nc.vector.reciprocal(rc[0:1, :, :], rs[0:1, :, :])
                    bc = bc_pool.tile([64, 2, 512], F32, tag="bc")
                    nc.gpsimd.partition_broadcast(bc[:], rc[0:1, :, :])
                    for sub in (0, 1):
                        half = slice(64 * sub, 64 * (sub + 1))
                        nc.vector.tensor_mul(
                            saT_sb[half, p, sq0:sq0 + 512],
                            av[0:64, sub, :], bc[:, sub, :])

                kq_proj(0)
                kq_proj(1)
                v_proj(0, 4)
                kq_proj(2)
                kq_proj(3)
                v_proj(4, 16)
                pools.pop("v_ps").release()
                pools.pop("kq_ps").release()
                get_pool("st_ps", 2); get_pool("av_ps", 2)
                for c in range(SCH):
                    attn_segment(c, 0)
                    attn_segment(c, 1)

              for pl in reversed(list(pools)):
                  pools.pop(pl).release()
              # ---------------- phase C: output projection ----------------
              with tc.tile_pool(name="o_ps", bufs=2, space="PSUM") as o_ps:
                for t in range(SKT):
                    ps = o_ps.tile([128, 1024], F32, tag="o")
                    for n in range(2):
                        nc.tensor.matmul(ps[:, 512 * n:512 * (n + 1)],
                                         saT_sb[:, 0, 128 * t:128 * (t + 1)],
                                         wo_sb[:, 0, 512 * n:512 * (n + 1)],
                                         start=True, stop=False)
                        nc.tensor.matmul(ps[:, 512 * n:512 * (n + 1)],
                                         saT_sb[:, 1, 128 * t:128 * (t + 1)],
                                         wo_sb[:, 1, 512 * n:512 * (n + 1)],
                                         start=False, stop=True)
                    ot = ostage.tile([128, 1024], F32, tag="ot")
                    if t % 2 == 0:
                        nc.vector.tensor_copy(ot[:], ps[:])
                    else:
                        nc.scalar.copy(ot[:], ps[:])
                    nc.sync.dma_start(out[128 * t:128 * (t + 1), :], ot[:])

    nc.compile()
    return nc


_CACHE = {}


def _build_runner():
    """Build the SPMD PJRT executable once; returns a dict with a jitted fn.

    Mirrors concourse.bass2jax.run_bass_via_pjrt but hoisted so repeated
    kernel() calls reuse the traced/compiled executable. No donation: the
    kernel DMA-writes every output element, so uninitialized output buffers
    are fine.
    """
    import jax
    from jax.sharding import Mesh, PartitionSpec
    from jax.experimental.shard_map import shard_map
    from concourse import bass2jax as b2j
    from concourse import mybir as _mybir

    if "runner" in _CACHE:
        return _CACHE["runner"]

    nc = _CACHE.get("nc")
    if nc is None:
        nc = _CACHE["nc"] = build_nc()

    b2j.install_neuronx_cc_hook()
    partition_name = (nc.partition_id_tensor.name
                      if nc.partition_id_tensor else None)

    in_names, out_names, out_avals = [], [], []
    for alloc in nc.m.functions[0].allocations:
        if not isinstance(alloc, _mybir.MemoryLocationSet):
            continue
        name = alloc.memorylocations[0].name
        if alloc.kind == "ExternalInput":
            if name != partition_name:
                in_names.append(name)
        elif alloc.kind == "ExternalOutput":
            out_names.append(name)
            out_avals.append(jax.core.ShapedArray(
                tuple(alloc.tensor_shape), _mybir.dt.np(alloc.dtype)))
    n_params = len(in_names)
    zero_out_shapes = [(a.shape, a.dtype) for a in out_avals]
    all_in_names = list(in_names) + list(out_names)
    if partition_name is not None:
        all_in_names.append(partition_name)

    def _body(*args):
        operands = list(args)
        if partition_name is not None:
            operands.append(b2j.partition_id_tensor())
        outs = b2j._bass_exec_p.bind(
            *operands,
            out_avals=tuple(out_avals),
            in_names=tuple(all_in_names),
            out_names=tuple(out_names),
            lowering_input_output_aliases=(),
            sim_require_finite=True,
            sim_require_nnan=True,
            nc=nc,
        )
        return tuple(outs)

    devices = jax.devices()[:N_CORES]
    mesh = Mesh(np.asarray(devices), ("core",))
    n_outs = len(out_names)
    in_specs = (PartitionSpec("core"),) * (n_params + n_outs)
    out_specs = (PartitionSpec("core"),) * n_outs
    fn = jax.jit(shard_map(_body, mesh=mesh, in_specs=in_specs,
                           out_specs=out_specs, check_rep=False),
                 keep_unused=True)
    runner = {
        "fn": fn,
        "in_names": in_names,
        "out_names": out_names,
        "out_avals": out_avals,
        "zero_out_shapes": zero_out_shapes,
        "mesh": mesh,
    }
    _CACHE["runner"] = runner
    return runner


def _run_spmd(in_maps):
    """Execute on 8 cores, returning list of per-core output dicts."""
    r = _build_runner()
    n_cores = N_CORES
    concat_in = [
        np.concatenate([np.asarray(in_maps[c][name]) for c in range(n_cores)],
                       axis=0)
        for name in r["in_names"]
    ]
    if "zeros" not in r:
        r["zeros"] = [np.zeros((n_cores * s[0], *s[1:]), d)
                      for s, d in r["zero_out_shapes"]]
    out_arrs = r["fn"](*concat_in, *r["zeros"])
    return [
        {name: np.asarray(out_arrs[i]).reshape(n_cores, *r["out_avals"][i].shape)[c]
         for i, name in enumerate(r["out_names"])}
        for c in range(n_cores)
    ]


def _prep_core_inputs(x, Wkqv, bkqv, Wo):
    """Host-side shard/pack. Returns (in_maps, host_bias) for 8 cores."""
    xT = [np.ascontiguousarray(x[b].T).astype(NP_BF16) for b in range(B)]
    per_g = []
    for g in range(4):
        h0 = 4 * g
        wkq = np.empty((E, 512), np.float32)
        for p in range(2):
            a, b_ = h0 + 2 * p, h0 + 2 * p + 1
            wkq[:, 256 * p:256 * p + 64] = Wkqv[a][:, 0:64]
            wkq[:, 256 * p + 64:256 * p + 128] = Wkqv[b_][:, 0:64]
            wkq[:, 256 * p + 128:256 * p + 192] = Wkqv[a][:, 64:128]
            wkq[:, 256 * p + 192:256 * p + 256] = Wkqv[b_][:, 64:128]
        wv = np.concatenate([Wkqv[h0 + h][:, 128:192] for h in range(HPC)],
                            axis=1)
        wog = Wo[256 * g:256 * (g + 1), :]
        bkq_arr = np.empty((128, 4), np.float32)
        for p in range(2):
            a, b_ = h0 + 2 * p, h0 + 2 * p + 1
            bkq_arr[0:64, 2 * p] = bkqv[a][0:64]
            bkq_arr[64:128, 2 * p] = bkqv[b_][0:64]
            bkq_arr[0:64, 2 * p + 1] = bkqv[a][64:128]
            bkq_arr[64:128, 2 * p + 1] = bkqv[b_][64:128]
        per_g.append({
            "wkq": wkq.astype(NP_BF16),
            "wv": wv.astype(NP_BF16),
            "wo": wog.astype(NP_BF16),
            "bkq": bkq_arr,
        })
    in_maps = []
    for c in range(N_CORES):
        b, g = c // 4, c % 4
        m = dict(per_g[g])
        m["xT"] = xT[b]
        in_maps.append(m)
    bv = np.concatenate([bkqv[h][128:192] for h in range(H)])
    return in_maps, bv


def kernel(x, Wkqv, bkqv, Wo, bo):
    x = np.asarray(x, np.float32)
    Wkqv = np.asarray(Wkqv, np.float32)
    bkqv = np.asarray(bkqv, np.float32)
    Wo = np.asarray(Wo, np.float32)
    bo = np.asarray(bo, np.float32)

    in_maps, bv = _prep_core_inputs(x, Wkqv, bkqv, Wo)
    results = _run_spmd(in_maps)
    partials = np.stack([results[c]["out"] for c in range(N_CORES)])
    partials = partials.reshape(B, 4, S, E).sum(axis=1)
    base = bv @ Wo + bo
    return (partials + base[None, None, :]).astype(np.float32)



# revision 4
# speedup vs baseline: 1.6711x; 1.6711x over previous
"""Causal self-attention (B=2, S=2048, E=1024, H=16, DH=64) on 8 trn2 cores.

Sharding: core c -> (batch b = c//4, head-group g = c%4, heads 4g..4g+3).
Data parallel over batch, tensor parallel over heads, row-sharded Wo;
partial outputs summed on host.

Per-core device kernel (bf16 matmuls, fp32 accumulation), emitted as a
single software pipeline over sq chunks c=0..3 so projection / output
matmuls fill the PE gaps of the ACT-bound attention loop (keeps HAM warm):

  prologue: kq chunk 0, v tiles 0..3
  for c: attn(c, p0), attn(c, p1), kq chunk c+1, v tiles, o chunk c

  attn(c, p): scores^T[sk,sq] = k q^T (2 heads row-packed in PE, K=64),
     P^T = exp(scores/8) (ACT; causal-trimmed, triu-masked diag),
     AV: saT_aug = v_aug^T @ P^T (row 64 = softmax row-sums via the ones
     column); normalization: reciprocal_approx_fast on the PSUM row-sum
     row, gpsimd partition-broadcast, fused DVE multiply eviction.

PSUM budget (8 banks, all pools persistent): proj 2 + st 4 + av 2.
State tiles (kqT / vaug / saT) ping-pong across reps (bufs=2 pools).
"""
import numpy as np
import ml_dtypes

import concourse.bass as bass
import concourse.bacc as bacc
import concourse.tile as tile
from concourse import mybir
from concourse.masks import make_upper_triangular

BF16 = mybir.dt.bfloat16
F32 = mybir.dt.float32
NP_BF16 = ml_dtypes.bfloat16

B, S, E, H, DH = 2, 2048, 1024, 16, 64
N_CORES = 8
HPC = 4          # heads per core
SCH = 4          # number of 512-wide sq chunks
SKT = 16         # number of 128-wide sk tiles
ET = 8           # number of 128-wide e tiles

Exp = mybir.ActivationFunctionType.Exp


def build_nc(reps=1):
    nc = bacc.Bacc(None, target_bir_lowering=False)

    xT = nc.dram_tensor("xT", [E, S], BF16, kind="ExternalInput")
    wkq = nc.dram_tensor("wkq", [E, 512], BF16, kind="ExternalInput")
    wv = nc.dram_tensor("wv", [E, 256], BF16, kind="ExternalInput")
    wo = nc.dram_tensor("wo", [256, E], BF16, kind="ExternalInput")
    bkq = nc.dram_tensor("bkq", [128, 4], F32, kind="ExternalInput")
    out = nc.dram_tensor("out", [S, E], BF16, kind="ExternalOutput")

    with tile.TileContext(nc) as tc:
        import contextlib
        with contextlib.ExitStack() as ctx:
            const = ctx.enter_context(tc.tile_pool(name="const", bufs=1))
            wpool = ctx.enter_context(tc.tile_pool(name="wpool", bufs=1))
            xpool = ctx.enter_context(tc.tile_pool(name="xpool", bufs=1))
            kqpool = ctx.enter_context(tc.tile_pool(name="kqpool", bufs=2))
            vpool = ctx.enter_context(tc.tile_pool(name="vpool", bufs=2))
            sapool = ctx.enter_context(tc.tile_pool(name="sapool", bufs=2))
            pt_pool = ctx.enter_context(tc.tile_pool(name="pt", bufs=6))
            bc_pool = ctx.enter_context(tc.tile_pool(name="bc", bufs=2))
            small = ctx.enter_context(tc.tile_pool(name="small", bufs=2))
            ostage = ctx.enter_context(tc.tile_pool(name="ostage", bufs=4))
            # persistent PSUM pools: proj(1 bank)x2 + st(2 banks)x2 +
            # av(2 banks)x1 = 8 banks
            proj_ps = ctx.enter_context(
                tc.tile_pool(name="proj_ps", bufs=2, space="PSUM"))
            st_ps = ctx.enter_context(
                tc.tile_pool(name="st_ps", bufs=2, space="PSUM"))
            av_ps = ctx.enter_context(
                tc.tile_pool(name="av_ps", bufs=1, space="PSUM"))

            bkq_sb = const.tile([128, 4], F32)
            nc.sync.dma_start(bkq_sb[:], bkq[:])
            triu2 = const.tile([128, 2, 128], BF16)
            make_upper_triangular(nc, triu2[:, 0, :], val=1.0, diag=True)
            make_upper_triangular(nc, triu2[:, 1, :], val=1.0, diag=True)

            # interleave weight/activation loads so the first kq matmuls can
            # start as soon as their e-tile arrives
            wkq_sb = wpool.tile([128, ET, 512], BF16)
            xT_sb = xpool.tile([128, ET, S], BF16)
            for e in range(ET):
                nc.sync.dma_start(wkq_sb[:, e, :], wkq[128 * e:128 * (e + 1), :])
                nc.sync.dma_start(xT_sb[:, e, :], xT[128 * e:128 * (e + 1), :])
            wv_sb = wpool.tile([128, ET, 256], BF16)
            nc.sync.dma_start(wv_sb[:], wv.rearrange("(n p) f -> p n f", p=128))
            wo_sb = wpool.tile([128, 2, E], BF16)
            nc.sync.dma_start(wo_sb[:], wo.rearrange("(n p) f -> p n f", p=128))

            for _rep in range(reps):
                kqT_sb = kqpool.tile([128, 4, S], BF16, tag="kqT")
                vaug_sb = vpool.tile([128, SKT, HPC, 65], BF16, tag="vaug")
                nc.vector.memset(vaug_sb[:, :, :, 64:65], 1.0)
                saT_sb = sapool.tile([128, 2, S], BF16, tag="saT")

                def kq_chunk(c):
                    for blk in range(4):
                        ps = proj_ps.tile([128, 512], F32, tag="proj")
                        for e in range(ET):
                            nc.tensor.matmul(
                                ps[:], wkq_sb[:, e, 128 * blk:128 * (blk + 1)],
                                xT_sb[:, e, 512 * c:512 * (c + 1)],
                                start=(e == 0), stop=(e == ET - 1))
                        nc.vector.tensor_scalar_add(
                            kqT_sb[:, blk, 512 * c:512 * (c + 1)], ps[:],
                            bkq_sb[:, blk:blk + 1])

                def v_chunk(c):
                    for t in range(4 * c, 4 * c + 4):
                        ps = proj_ps.tile([128, 256], F32, tag="proj")
                        for e in range(ET):
                            nc.tensor.matmul(
                                ps[:], xT_sb[:, e, 128 * t:128 * (t + 1)],
                                wv_sb[:, e, :],
                                start=(e == 0), stop=(e == ET - 1))
                        nc.vector.tensor_copy(
                            vaug_sb[:, t, :, 0:64],
                            ps[:].rearrange("p (h d) -> p h d", h=HPC))

                def attn_segment(c, p):
                    sq0 = 512 * c
                    kblk, qblk = 2 * p, 2 * p + 1
                    av = av_ps.tile([65, 2, 512], F32, tag="av")
                    nj = 4 * c + 4
                    for j in range(nj):
                        r = j - 4 * c
                        diag = r >= 0
                        off = 128 * r if diag else 0
                        w = 512 - off
                        st = st_ps.tile([128, 2, 512], F32, tag="st")
                        nc.tensor.matmul(
                            st[:, 0, 0:w],
                            kqT_sb[0:64, kblk, 128 * j:128 * (j + 1)],
                            kqT_sb[0:64, qblk, sq0 + off:sq0 + 512],
                            start=True, stop=True, tile_position=(0, 0))
                        nc.tensor.matmul(
                            st[:, 1, 0:w],
                            kqT_sb[64:128, kblk, 128 * j:128 * (j + 1)],
                            kqT_sb[64:128, qblk, sq0 + off:sq0 + 512],
                            start=True, stop=True, tile_position=(64, 0))
                        pt = pt_pool.tile([128, 2, 512], BF16, tag="pt")
                        nc.scalar.activation(pt[:, :, off:512],
                                             st[:, :, 0:w],
                                             Exp, scale=0.125)
                        if diag:
                            nc.vector.tensor_mul(
                                pt[:, :, off:off + 128],
                                pt[:, :, off:off + 128], triu2[:])
                        nc.tensor.matmul(av[:, 0, off:512],
                                         vaug_sb[:, j, 2 * p, :],
                                         pt[:, 0, off:512], start=(j == 0),
                                         stop=(j == nj - 1))
                        nc.tensor.matmul(av[:, 1, off:512],
                                         vaug_sb[:, j, 2 * p + 1, :],
                                         pt[:, 1, off:512], start=(j == 0),
                                         stop=(j == nj - 1))
                    # normalization + fused eviction for (p, c)
                    rs = small.tile([1, 2, 512], F32, tag="rs")
                    nc.vector.tensor_copy(rs[0:1, :, :], av[64:65, :, :])
                    rc = small.tile([1, 2, 512], F32, tag="rc")
                    nc.vector.reciprocal_approx_fast(rc[0:1, :, :],
                                                     rs[0:1, :, :])
                    bc = bc_pool.tile([64, 2, 512], F32, tag="bc")
                    nc.gpsimd.partition_broadcast(bc[:], rc[0:1, :, :])
                    for sub in (0, 1):
                        half = slice(64 * sub, 64 * (sub + 1))
                        nc.vector.tensor_mul(
                            saT_sb[half, p, sq0:sq0 + 512],
                            av[0:64, sub, :], bc[:, sub, :])

                def o_chunk(c):
                    for t in range(4 * c, 4 * c + 4):
                        ot = ostage.tile([128, 2, 512], BF16, tag="ot")
                        for n in range(2):
                            ps = proj_ps.tile([128, 512], F32, tag="proj")
                            nc.tensor.matmul(
                                ps[:], saT_sb[:, 0, 128 * t:128 * (t + 1)],
                                wo_sb[:, 0, 512 * n:512 * (n + 1)],
                                start=True, stop=False)
                            nc.tensor.matmul(
                                ps[:], saT_sb[:, 1, 128 * t:128 * (t + 1)],
                                wo_sb[:, 1, 512 * n:512 * (n + 1)],
                                start=False, stop=True)
                            nc.vector.tensor_copy(ot[:, n, :], ps[:])
                        nc.sync.dma_start(
                            out[128 * t:128 * (t + 1), :],
                            ot[:].rearrange("p a b -> p (a b)"))

                kq_chunk(0)
                v_chunk(0)
                for c in range(SCH):
                    attn_segment(c, 0)
                    attn_segment(c, 1)
                    if c + 1 < SCH:
                        kq_chunk(c + 1)
                        v_chunk(c + 1)
                    o_chunk(c)

    nc.compile()
    return nc


_CACHE = {}


def _build_runner():
    """Build the SPMD PJRT executable once; returns a dict with a jitted fn.

    Mirrors concourse.bass2jax.run_bass_via_pjrt but hoisted so repeated
    kernel() calls reuse the traced/compiled executable. No donation: the
    kernel DMA-writes every output element, so uninitialized output buffers
    are fine.
    """
    import jax
    from jax.sharding import Mesh, PartitionSpec
    from jax.experimental.shard_map import shard_map
    from concourse import bass2jax as b2j
    from concourse import mybir as _mybir

    if "runner" in _CACHE:
        return _CACHE["runner"]

    nc = _CACHE.get("nc")
    if nc is None:
        nc = _CACHE["nc"] = build_nc()

    b2j.install_neuronx_cc_hook()
    partition_name = (nc.partition_id_tensor.name
                      if nc.partition_id_tensor else None)

    in_names, out_names, out_avals = [], [], []
    for alloc in nc.m.functions[0].allocations:
        if not isinstance(alloc, _mybir.MemoryLocationSet):
            continue
        name = alloc.memorylocations[0].name
        if alloc.kind == "ExternalInput":
            if name != partition_name:
                in_names.append(name)
        elif alloc.kind == "ExternalOutput":
            out_names.append(name)
            out_avals.append(jax.core.ShapedArray(
                tuple(alloc.tensor_shape), _mybir.dt.np(alloc.dtype)))
    n_params = len(in_names)
    zero_out_shapes = [(a.shape, a.dtype) for a in out_avals]
    all_in_names = list(in_names) + list(out_names)
    if partition_name is not None:
        all_in_names.append(partition_name)

    def _body(*args):
        operands = list(args)
        if partition_name is not None:
            operands.append(b2j.partition_id_tensor())
        outs = b2j._bass_exec_p.bind(
            *operands,
            out_avals=tuple(out_avals),
            in_names=tuple(all_in_names),
            out_names=tuple(out_names),
            lowering_input_output_aliases=(),
            sim_require_finite=True,
            sim_require_nnan=True,
            nc=nc,
        )
        return tuple(outs)

    devices = jax.devices()[:N_CORES]
    mesh = Mesh(np.asarray(devices), ("core",))
    n_outs = len(out_names)
    in_specs = (PartitionSpec("core"),) * (n_params + n_outs)
    out_specs = (PartitionSpec("core"),) * n_outs
    fn = jax.jit(shard_map(_body, mesh=mesh, in_specs=in_specs,
                           out_specs=out_specs, check_rep=False),
                 keep_unused=True)
    runner = {
        "fn": fn,
        "in_names": in_names,
        "out_names": out_names,
        "out_avals": out_avals,
        "zero_out_shapes": zero_out_shapes,
        "mesh": mesh,
    }
    _CACHE["runner"] = runner
    return runner


def _run_spmd(in_maps):
    """Execute on 8 cores, returning list of per-core output dicts."""
    r = _build_runner()
    n_cores = N_CORES
    concat_in = [
        np.concatenate([np.asarray(in_maps[c][name]) for c in range(n_cores)],
                       axis=0)
        for name in r["in_names"]
    ]
    if "zeros" not in r:
        r["zeros"] = [np.zeros((n_cores * s[0], *s[1:]), d)
                      for s, d in r["zero_out_shapes"]]
    out_arrs = r["fn"](*concat_in, *r["zeros"])
    return [
        {name: np.asarray(out_arrs[i]).reshape(n_cores, *r["out_avals"][i].shape)[c]
         for i, name in enumerate(r["out_names"])}
        for c in range(n_cores)
    ]


def _prep_core_inputs(x, Wkqv, bkqv, Wo):
    """Host-side shard/pack. Returns (in_maps, host_bias) for 8 cores."""
    xT = [np.ascontiguousarray(x[b].T).astype(NP_BF16) for b in range(B)]
    per_g = []
    for g in range(4):
        h0 = 4 * g
        wkq = np.empty((E, 512), np.float32)
        for p in range(2):
            a, b_ = h0 + 2 * p, h0 + 2 * p + 1
            wkq[:, 256 * p:256 * p + 64] = Wkqv[a][:, 0:64]
            wkq[:, 256 * p + 64:256 * p + 128] = Wkqv[b_][:, 0:64]
            wkq[:, 256 * p + 128:256 * p + 192] = Wkqv[a][:, 64:128]
            wkq[:, 256 * p + 192:256 * p + 256] = Wkqv[b_][:, 64:128]
        wv = np.concatenate([Wkqv[h0 + h][:, 128:192] for h in range(HPC)],
                            axis=1)
        wog = Wo[256 * g:256 * (g + 1), :]
        bkq_arr = np.empty((128, 4), np.float32)
        for p in range(2):
            a, b_ = h0 + 2 * p, h0 + 2 * p + 1
            bkq_arr[0:64, 2 * p] = bkqv[a][0:64]
            bkq_arr[64:128, 2 * p] = bkqv[b_][0:64]
            bkq_arr[0:64, 2 * p + 1] = bkqv[a][64:128]
            bkq_arr[64:128, 2 * p + 1] = bkqv[b_][64:128]
        per_g.append({
            "wkq": wkq.astype(NP_BF16),
            "wv": wv.astype(NP_BF16),
            "wo": wog.astype(NP_BF16),
            "bkq": bkq_arr,
        })
    in_maps = []
    for c in range(N_CORES):
        b, g = c // 4, c % 4
        m = dict(per_g[g])
        m["xT"] = xT[b]
        in_maps.append(m)
    bv = np.concatenate([bkqv[h][128:192] for h in range(H)])
    return in_maps, bv


def kernel(x, Wkqv, bkqv, Wo, bo):
    x = np.asarray(x, np.float32)
    Wkqv = np.asarray(Wkqv, np.float32)
    bkqv = np.asarray(bkqv, np.float32)
    Wo = np.asarray(Wo, np.float32)
    bo = np.asarray(bo, np.float32)

    in_maps, bv = _prep_core_inputs(x, Wkqv, bkqv, Wo)
    results = _run_spmd(in_maps)
    partials = np.stack([results[c]["out"].astype(np.float32)
                         for c in range(N_CORES)])
    partials = partials.reshape(B, 4, S, E).sum(axis=1)
    base = bv @ Wo + bo
    return (partials + base[None, None, :]).astype(np.float32)


# revision 19
# speedup vs baseline: 28.1033x; 16.8173x over previous
"""Causal self-attention (B=2, S=2048, E=1024, H=16, DH=64) on 8 trn2 cores.

Sharding: core c -> (batch b = c//4, head-group g = c%4, heads 4g..4g+3).
Data parallel over batch, tensor parallel over heads, row-sharded Wo;
partial outputs summed on host.

Per-core device kernel (bf16 matmuls, fp32 accumulation), emitted as a
single software pipeline over sq chunks c=0..3 so projection / output
matmuls fill the PE gaps of the ACT-bound attention loop (keeps HAM warm):

  prologue: kq chunk 0, v tiles 0..3
  for rep: for c: attn(c, p0), attn(c, p1),
           kq/v chunk c+1 (or next rep's chunk 0), o chunk c

  attn(c, p): scores^T[sk,sq] = k q^T (2 heads row-packed in PE, K=64),
     P^T = exp(scores/8) (ACT; causal-trimmed, triu-masked diag),
     AV: saT_aug = v_aug^T @ P^T (row 64 = softmax row-sums via the ones
     column); normalization: reciprocal_approx_fast on the row sums,
     gpsimd partition-broadcast, fused DVE multiply eviction.
  Score tiles are emitted one step ahead of the exp/AV consumer (the
  lookahead crosses the p0->p1 boundary); the normalize is split into a
  [0:384) strip (emitted under the last diagonal AVs) and a [384:512)
  tail whose av-PSUM reads are snapshotted to SBUF so the next segment's
  AV matmuls start without waiting for the reciprocal chain.

PSUM budget (8 banks, all pools persistent): proj 2 + st 4 + av 2.
State tiles (kqT / vaug / saT) ping-pong across reps (bufs=2 pools);
next-rep kq/v projections are emitted before o(3) so the proj-pool FIFO
rotation cannot stall the rep boundary.
"""
import numpy as np
import ml_dtypes

import concourse.bass as bass
import concourse.bacc as bacc
import concourse.tile as tile
from concourse import mybir
from concourse.masks import make_upper_triangular

BF16 = mybir.dt.bfloat16
F32 = mybir.dt.float32
NP_BF16 = ml_dtypes.bfloat16

B, S, E, H, DH = 2, 2048, 1024, 16, 64
N_CORES = 8
HPC = 4          # heads per core
SCH = 4          # number of 512-wide sq chunks
SKT = 16         # number of 128-wide sk tiles
ET = 8           # number of 128-wide e tiles

Exp = mybir.ActivationFunctionType.Exp


def build_nc(reps=1):
    nc = bacc.Bacc(None, target_bir_lowering=False)

    xT = nc.dram_tensor("xT", [E, S], BF16, kind="ExternalInput")
    wkq = nc.dram_tensor("wkq", [E, 512], BF16, kind="ExternalInput")
    wv = nc.dram_tensor("wv", [E, 256], BF16, kind="ExternalInput")
    wo = nc.dram_tensor("wo", [256, E], BF16, kind="ExternalInput")
    bkq = nc.dram_tensor("bkq", [128, 4], F32, kind="ExternalInput")
    out = nc.dram_tensor("out", [S, E], BF16, kind="ExternalOutput")

    with tile.TileContext(nc) as tc:
        import contextlib
        with contextlib.ExitStack() as ctx:
            const = ctx.enter_context(tc.tile_pool(name="const", bufs=1))
            wpool = ctx.enter_context(tc.tile_pool(name="wpool", bufs=1))
            xpool = ctx.enter_context(tc.tile_pool(name="xpool", bufs=1))
            kqpool = ctx.enter_context(tc.tile_pool(name="kqpool", bufs=2))
            vpool = ctx.enter_context(tc.tile_pool(name="vpool", bufs=2))
            sapool = ctx.enter_context(tc.tile_pool(name="sapool", bufs=2))
            pt_pool = ctx.enter_context(tc.tile_pool(name="pt", bufs=8))
            bc_pool = ctx.enter_context(tc.tile_pool(name="bc", bufs=2))
            small = ctx.enter_context(tc.tile_pool(name="small", bufs=2))
            ostage = ctx.enter_context(tc.tile_pool(name="ostage", bufs=6))
            # persistent PSUM pools: proj(1 bank)x2 + st(2 banks)x2 +
            # av(2 banks)x1 = 8 banks
            proj_ps = ctx.enter_context(
                tc.tile_pool(name="proj_ps", bufs=2, space="PSUM"))
            st_ps = ctx.enter_context(
                tc.tile_pool(name="st_ps", bufs=2, space="PSUM"))
            av_ps = ctx.enter_context(
                tc.tile_pool(name="av_ps", bufs=1, space="PSUM"))

            bkq_sb = const.tile([128, 4], F32)
            nc.sync.dma_start(bkq_sb[:], bkq[:])
            triu2 = const.tile([128, 2, 128], BF16)
            make_upper_triangular(nc, triu2[:, 0, :], val=1.0, diag=True)
            make_upper_triangular(nc, triu2[:, 1, :], val=1.0, diag=True)

            # interleave weight/activation loads so the first kq matmuls can
            # start as soon as their e-tile arrives
            wkq_sb = wpool.tile([128, ET, 512], BF16)
            xT_sb = xpool.tile([128, ET, S], BF16)
            for e in range(ET):
                nc.sync.dma_start(wkq_sb[:, e, :], wkq[128 * e:128 * (e + 1), :])
                nc.sync.dma_start(xT_sb[:, e, :], xT[128 * e:128 * (e + 1), :])
            wv_sb = wpool.tile([128, ET, 256], BF16)
            nc.sync.dma_start(wv_sb[:], wv.rearrange("(n p) f -> p n f", p=128))
            wo_sb = wpool.tile([128, 2, E], BF16)
            nc.sync.dma_start(wo_sb[:], wo.rearrange("(n p) f -> p n f", p=128))

            # per-rep state tiles, allocated lazily so rep r+1's projection
            # matmuls can be emitted (and scheduled) before rep r's tail
            states = []

            def ensure_state(r):
                while len(states) <= r:
                    kqT_sb = kqpool.tile([128, 4, S], BF16, tag="kqT")
                    vaug_sb = vpool.tile([128, SKT, HPC, 65], BF16,
                                         tag="vaug")
                    nc.vector.memset(vaug_sb[:, :, :, 64:65], 1.0)
                    saT_sb = sapool.tile([128, 2, S], BF16, tag="saT")
                    states.append((kqT_sb, vaug_sb, saT_sb))

            def kq_chunk(r, c):
                kqT_sb = states[r][0]
                for blk in range(4):
                    ps = proj_ps.tile([128, 512], F32, tag="proj")
                    for e in range(ET):
                        nc.tensor.matmul(
                            ps[:], wkq_sb[:, e, 128 * blk:128 * (blk + 1)],
                            xT_sb[:, e, 512 * c:512 * (c + 1)],
                            start=(e == 0), stop=(e == ET - 1))
                    nc.vector.tensor_scalar_add(
                        kqT_sb[:, blk, 512 * c:512 * (c + 1)], ps[:],
                        bkq_sb[:, blk:blk + 1])

            def v_chunk(r, c):
                vaug_sb = states[r][1]
                for t in range(4 * c, 4 * c + 4):
                    ps = proj_ps.tile([128, 256], F32, tag="proj")
                    for e in range(ET):
                        nc.tensor.matmul(
                            ps[:], xT_sb[:, e, 128 * t:128 * (t + 1)],
                            wv_sb[:, e, :],
                            start=(e == 0), stop=(e == ET - 1))
                    nc.vector.tensor_copy(
                        vaug_sb[:, t, :, 0:64],
                        ps[:].rearrange("p (h d) -> p h d", h=HPC))

            def normalize(av, saT_sb, p, sq0, lo, hi, copy_out=False):
                # saT[:, p, sq0+lo : sq0+hi] = av[0:64] / rowsum  for the
                # column strip [lo:hi) whose rowsum row is final.
                # copy_out: snapshot the strip to SBUF first so the av PSUM
                # banks free as soon as the two copies retire (next
                # segment's AV matmuls don't wait for the recip chain).
                rs = small.tile([1, 2, 512], F32, tag="rs", bufs=4)
                nc.vector.tensor_copy(rs[0:1, :, 0:hi - lo],
                                      av[64:65, :, lo:hi])
                if copy_out:
                    avt = small.tile([64, 2, 128], F32, tag="avt", bufs=4)
                    nc.vector.tensor_copy(avt[:, :, 0:hi - lo],
                                          av[0:64, :, lo:hi])
                    sa_src = lambda sub: avt[:, sub, 0:hi - lo]
                else:
                    sa_src = lambda sub: av[0:64, sub, lo:hi]
                rc = small.tile([1, 2, 512], F32, tag="rc", bufs=4)
                nc.vector.reciprocal_approx_fast(rc[0:1, :, 0:hi - lo],
                                                 rs[0:1, :, 0:hi - lo])
                bc = bc_pool.tile([64, 2, 512], F32, tag="bc", bufs=4)
                nc.gpsimd.partition_broadcast(bc[:, :, 0:hi - lo],
                                              rc[0:1, :, 0:hi - lo])
                for sub in (0, 1):
                    half = slice(64 * sub, 64 * (sub + 1))
                    nc.vector.tensor_mul(
                        saT_sb[half, p, sq0 + lo:sq0 + hi],
                        sa_src(sub), bc[:, sub, 0:hi - lo])

            # ---- globally software-pipelined attention ----
            # scores are emitted one step ahead of the exp/AV consumer and
            # the lookahead crosses segment / chunk / rep boundaries, so the
            # PE never drains at a segment start waiting for the first exp.
            pending = []    # emitted scores not yet consumed

            def emit_scores(r, c, p, j):
                ensure_state(r)
                kqT_sb = states[r][0]
                sq0 = 512 * c
                kblk, qblk = 2 * p, 2 * p + 1
                rr = j - 4 * c
                off = 128 * rr if rr >= 0 else 0
                w = 512 - off
                st = st_ps.tile([128, 2, 512], F32, tag="st")
                nc.tensor.matmul(
                    st[:, 0, 0:w],
                    kqT_sb[0:64, kblk, 128 * j:128 * (j + 1)],
                    kqT_sb[0:64, qblk, sq0 + off:sq0 + 512],
                    start=True, stop=True, tile_position=(0, 0))
                nc.tensor.matmul(
                    st[:, 1, 0:w],
                    kqT_sb[64:128, kblk, 128 * j:128 * (j + 1)],
                    kqT_sb[64:128, qblk, sq0 + off:sq0 + 512],
                    start=True, stop=True, tile_position=(64, 0))
                pending.append((st, off, w))

            def attn_segment(r, c, p, scout):
                """Consume this segment's nj pre-ordered score tiles; scout()
                emits the next score tile from the global stream."""
                kqT_sb, vaug_sb, saT_sb = states[r]
                sq0 = 512 * c
                av = av_ps.tile([65, 2, 512], F32, tag="av")
                nj = 4 * c + 4
                for j in range(nj):
                    rr = j - 4 * c
                    if not pending:
                        scout(force=True)
                    st, off, w = pending.pop(0)
                    pt = pt_pool.tile([128, 2, 512], BF16, tag="pt")
                    nc.scalar.activation(pt[:, :, off:512],
                                         st[:, :, 0:w],
                                         Exp, scale=0.125)
                    scout()
                    if rr >= 0:
                        nc.vector.tensor_mul(
                            pt[:, :, off:off + 128],
                            pt[:, :, off:off + 128], triu2[:])
                    nc.tensor.matmul(av[:, 0, off:512],
                                     vaug_sb[:, j, 2 * p, :],
                                     pt[:, 0, off:512], start=(j == 0),
                                     stop=(j == nj - 1))
                    nc.tensor.matmul(av[:, 1, off:512],
                                     vaug_sb[:, j, 2 * p + 1, :],
                                     pt[:, 1, off:512], start=(j == 0),
                                     stop=(j == nj - 1))
                    if rr == 2:
                        # rowsum for sq strip [0:384) is final once the
                        # r<=2 diagonal AVs are in: normalize early so the
                        # tail chain only covers the last 128 columns
                        normalize(av, saT_sb, p, sq0, 0, 384)
                normalize(av, saT_sb, p, sq0, 384, 512, copy_out=True)

            def o_chunk(r, c):
                saT_sb = states[r][2]
                for t in range(4 * c, 4 * c + 4):
                    ot = ostage.tile([128, 2, 512], BF16, tag="ot")
                    ps0 = proj_ps.tile([128, 512], F32, tag="proj")
                    ps1 = proj_ps.tile([128, 512], F32, tag="proj")
                    nc.tensor.matmul(ps0[:],
                                     saT_sb[:, 0, 128 * t:128 * (t + 1)],
                                     wo_sb[:, 0, 0:512],
                                     start=True, stop=False)
                    nc.tensor.matmul(ps1[:],
                                     saT_sb[:, 0, 128 * t:128 * (t + 1)],
                                     wo_sb[:, 0, 512:1024],
                                     start=True, stop=False)
                    nc.tensor.matmul(ps0[:],
                                     saT_sb[:, 1, 128 * t:128 * (t + 1)],
                                     wo_sb[:, 1, 0:512],
                                     start=False, stop=True)
                    nc.tensor.matmul(ps1[:],
                                     saT_sb[:, 1, 128 * t:128 * (t + 1)],
                                     wo_sb[:, 1, 512:1024],
                                     start=False, stop=True)
                    nc.vector.tensor_copy(ot[:, 0, :], ps0[:])
                    nc.vector.tensor_copy(ot[:, 1, :], ps1[:])
                    nc.sync.dma_start(
                        out[128 * t:128 * (t + 1), :],
                        ot[:].rearrange("p a b -> p (a b)"))

            ensure_state(0)
            kq_chunk(0, 0)
            v_chunk(0, 0)

            # global score-tile stream in consumption order
            def tile_stream():
                for rep in range(reps):
                    for c in range(SCH):
                        for p in (0, 1):
                            for j in range(4 * c + 4):
                                yield (rep, c, p, j)
            stream = tile_stream()
            stream_peek = [next(stream, None)]

            for rep in range(reps):
                for c in range(SCH):
                    # prefetch may cross the p0->p1 boundary (same chunk's
                    # kqT already written) but NOT into the next chunk,
                    # whose kqT writes are emitted after this attention
                    def scout(force=False, _rc=(rep, c)):
                        nxt = stream_peek[0]
                        if nxt is None:
                            return
                        if force or (nxt[0], nxt[1]) == _rc:
                            emit_scores(*nxt)
                            stream_peek[0] = next(stream, None)
                    attn_segment(rep, c, 0, scout)
                    attn_segment(rep, c, 1, scout)
                    if c + 1 < SCH:
                        kq_chunk(rep, c + 1)
                        v_chunk(rep, c + 1)
                    elif rep + 1 < reps:
                        ensure_state(rep + 1)
                        kq_chunk(rep + 1, 0)
                        v_chunk(rep + 1, 0)
                    o_chunk(rep, c)

    nc.compile()
    return nc


_CACHE = {}


def _build_runner():
    """Build the SPMD PJRT executable once; returns a dict with a jitted fn.

    Mirrors concourse.bass2jax.run_bass_via_pjrt but hoisted so repeated
    kernel() calls reuse the traced/compiled executable. No donation: the
    kernel DMA-writes every output element, so uninitialized output buffers
    are fine.
    """
    import jax
    from jax.sharding import Mesh, PartitionSpec
    from jax.experimental.shard_map import shard_map
    from concourse import bass2jax as b2j
    from concourse import mybir as _mybir

    if "runner" in _CACHE:
        return _CACHE["runner"]

    nc = _CACHE.get("nc")
    if nc is None:
        nc = _CACHE["nc"] = build_nc()

    b2j.install_neuronx_cc_hook()
    partition_name = (nc.partition_id_tensor.name
                      if nc.partition_id_tensor else None)

    in_names, out_names, out_avals = [], [], []
    for alloc in nc.m.functions[0].allocations:
        if not isinstance(alloc, _mybir.MemoryLocationSet):
            continue
        name = alloc.memorylocations[0].name
        if alloc.kind == "ExternalInput":
            if name != partition_name:
                in_names.append(name)
        elif alloc.kind == "ExternalOutput":
            out_names.append(name)
            out_avals.append(jax.core.ShapedArray(
                tuple(alloc.tensor_shape), _mybir.dt.np(alloc.dtype)))
    n_params = len(in_names)
    zero_out_shapes = [(a.shape, a.dtype) for a in out_avals]
    all_in_names = list(in_names) + list(out_names)
    if partition_name is not None:
        all_in_names.append(partition_name)

    def _body(*args):
        operands = list(args)
        if partition_name is not None:
            operands.append(b2j.partition_id_tensor())
        outs = b2j._bass_exec_p.bind(
            *operands,
            out_avals=tuple(out_avals),
            in_names=tuple(all_in_names),
            out_names=tuple(out_names),
            lowering_input_output_aliases=(),
            sim_require_finite=True,
            sim_require_nnan=True,
            nc=nc,
        )
        return tuple(outs)

    devices = jax.devices()[:N_CORES]
    mesh = Mesh(np.asarray(devices), ("core",))
    n_outs = len(out_names)
    in_specs = (PartitionSpec("core"),) * (n_params + n_outs)
    out_specs = (PartitionSpec("core"),) * n_outs
    fn = jax.jit(shard_map(_body, mesh=mesh, in_specs=in_specs,
                           out_specs=out_specs, check_rep=False),
                 keep_unused=True)
    runner = {
        "fn": fn,
        "in_names": in_names,
        "out_names": out_names,
        "out_avals": out_avals,
        "zero_out_shapes": zero_out_shapes,
        "mesh": mesh,
    }
    _CACHE["runner"] = runner
    return runner


def _run_spmd(in_maps):
    """Execute on 8 cores, returning list of per-core output dicts."""
    r = _build_runner()
    n_cores = N_CORES
    concat_in = [
        np.concatenate([np.asarray(in_maps[c][name]) for c in range(n_cores)],
                       axis=0)
        for name in r["in_names"]
    ]
    if "zeros" not in r:
        r["zeros"] = [np.zeros((n_cores * s[0], *s[1:]), d)
                      for s, d in r["zero_out_shapes"]]
    out_arrs = r["fn"](*concat_in, *r["zeros"])
    return [
        {name: np.asarray(out_arrs[i]).reshape(n_cores, *r["out_avals"][i].shape)[c]
         for i, name in enumerate(r["out_names"])}
        for c in range(n_cores)
    ]


def _prep_core_inputs(x, Wkqv, bkqv, Wo):
    """Host-side shard/pack. Returns (in_maps, host_bias) for 8 cores."""
    xT = [np.ascontiguousarray(x[b].T).astype(NP_BF16) for b in range(B)]
    per_g = []
    for g in range(4):
        h0 = 4 * g
        wkq = np.empty((E, 512), np.float32)
        for p in range(2):
            a, b_ = h0 + 2 * p, h0 + 2 * p + 1
            wkq[:, 256 * p:256 * p + 64] = Wkqv[a][:, 0:64]
            wkq[:, 256 * p + 64:256 * p + 128] = Wkqv[b_][:, 0:64]
            wkq[:, 256 * p + 128:256 * p + 192] = Wkqv[a][:, 64:128]
            wkq[:, 256 * p + 192:256 * p + 256] = Wkqv[b_][:, 64:128]
        wv = np.concatenate([Wkqv[h0 + h][:, 128:192] for h in range(HPC)],
                            axis=1)
        wog = Wo[256 * g:256 * (g + 1), :]
        bkq_arr = np.empty((128, 4), np.float32)
        for p in range(2):
            a, b_ = h0 + 2 * p, h0 + 2 * p + 1
            bkq_arr[0:64, 2 * p] = bkqv[a][0:64]
            bkq_arr[64:128, 2 * p] = bkqv[b_][0:64]
            bkq_arr[0:64, 2 * p + 1] = bkqv[a][64:128]
            bkq_arr[64:128, 2 * p + 1] = bkqv[b_][64:128]
        per_g.append({
            "wkq": wkq.astype(NP_BF16),
            "wv": wv.astype(NP_BF16),
            "wo": wog.astype(NP_BF16),
            "bkq": bkq_arr,
        })
    in_maps = []
    for c in range(N_CORES):
        b, g = c // 4, c % 4
        m = dict(per_g[g])
        m["xT"] = xT[b]
        in_maps.append(m)
    bv = np.concatenate([bkqv[h][128:192] for h in range(H)])
    return in_maps, bv


def kernel(x, Wkqv, bkqv, Wo, bo):
    x = np.asarray(x, np.float32)
    Wkqv = np.asarray(Wkqv, np.float32)
    bkqv = np.asarray(bkqv, np.float32)
    Wo = np.asarray(Wo, np.float32)
    bo = np.asarray(bo, np.float32)

    in_maps, bv = _prep_core_inputs(x, Wkqv, bkqv, Wo)
    results = _run_spmd(in_maps)
    partials = np.stack([results[c]["out"].astype(np.float32)
                         for c in range(N_CORES)])
    partials = partials.reshape(B, 4, S, E).sum(axis=1)
    base = bv @ Wo + bo
    return (partials + base[None, None, :]).astype(np.float32)
